# revision 33
# baseline (speedup 1.0000x reference)
"""Trainium2 Bass kernel for nn_Encoder_69939247448302 (fairseq-style conv encoder
+ Gumbel VQ eval path). Data-parallel over batch across 8 NeuronCores.

Per-core pipeline (B_local=8, channel-major layouts, T on the free dim):
  conv1/2/3 as stacked-shift accumulated matmuls (full-rank contractions),
  qe matmul, wp logits computed t-major (q-tile as the stationary operand),
  exact f32 argmax via DVE max/max_index, codebook + (de_w@codebook) product
  tables gathered by index via SWDGE dma_gather, pooled = partition_all_reduce
  max over gathered product-table sums.
"""
import sys

sys.path.insert(0, "/opt/trn_rl_repo")

import numpy as np

B, T, C = 64, 2048, 9
NCORES = 8
BL = B // NCORES                 # 8 batch elems per core
T1, T2, T3 = 2025, 2010, 2003    # conv output lengths (VALID, K=24/16/8)
G, V, VD = 2, 320, 64
TPAD = 2048                      # padded time for k staging / gathers
NT = TPAD // 128                 # 16 t-tiles per batch
LASTN = T3 - 15 * 128            # 83 valid rows in last t-tile

_built = {}
_DBG_STAGES = 99   # debug: 1=conv1, 2=+conv2, 3=+conv3, 4=+qe, 5=+wp/argmax, 6=+gathers, 7=all
_DBG_BL = BL
_DBG_SUB = 9   # sub-stages within stage 6
_DBG_DUMP = False


def _tiles(total, step=512):
    out = []
    t0 = 0
    while t0 < total:
        out.append((t0, min(step, total - t0)))
        t0 += step
    return out


def _build(with_wpb):
    import bass_rust
    import concourse.bass as bass
    import concourse.tile as tile
    from concourse import bacc, mybir
    import concourse.bass_isa as bass_isa
    from contextlib import ExitStack

    f32 = mybir.dt.float32
    u16 = mybir.dt.uint16
    i16 = mybir.dt.int16
    i32 = mybir.dt.int32
    RELU = mybir.ActivationFunctionType.Relu
    X = mybir.AxisListType.X

    nc = bacc.Bacc("TRN2", target_bir_lowering=False, debug=False,
                   enable_asserts=False, num_devices=NCORES)

    din = lambda name, shape, dt=f32: nc.dram_tensor(name, shape, dt, kind="ExternalInput").ap()
    xs = din("xs", [BL, 1, T, C])
    z1w0 = din("z1w0", [126, 32])
    z1w1 = din("z1w1", [90, 32])
    z2w = [din(f"z2w{g}", [128, 64]) for g in range(4)]
    z3w = [din(f"z3w{g}", [128, 96]) for g in range(4)]
    qewT = din("qewT", [96, 128])
    wpT = din("wpT", [128, 640])
    cbq = din("cbq", [641, VD])
    p0t = din("p0t", [321, 128])
    p1t = din("p1t", [321, 128])
    c1b = din("c1b", [32, 1])
    c2b = din("c2b", [64, 1])
    c3b = din("c3b", [96, 1])
    qeb = din("qeb", [128, 1])
    kpad = din("kpad", [128, 2], u16)
    ident = din("ident", [128, 128])
    wpbr = din("wpbr", [128, 640]) if with_wpb else None

    pooled = nc.dram_tensor("pooled", [BL, 96], f32, kind="ExternalOutput").ap()
    quant = nc.dram_tensor("quant", [BL, T3, 128], f32, kind="ExternalOutput").ap()
    targets = nc.dram_tensor("targets", [BL, T3, G], i32, kind="ExternalOutput").ap()
    kdram = nc.dram_tensor("kdram", [BL, TPAD, G], u16, kind="Internal").ap()
    dbg = nc.dram_tensor("dbg", [4, 128, 16, 128], f32, kind="ExternalOutput").ap() if _DBG_DUMP else None

    with tile.TileContext(nc) as tc:
        with ExitStack() as ctx:
            const = ctx.enter_context(tc.tile_pool(name="const", bufs=1))
            wk = ctx.enter_context(tc.tile_pool(name="wk", bufs=2))
            lsp = ctx.enter_context(tc.tile_pool(name="lsp", bufs=4))
            sm = ctx.enter_context(tc.tile_pool(name="sm", bufs=4))
            dpool = ctx.enter_context(tc.tile_pool(name="dpool", bufs=8))
            dgp = ctx.enter_context(tc.tile_pool(name="dgp", bufs=3))
            xp = ctx.enter_context(tc.tile_pool(name="xp", bufs=3))
            dmax_tiles = []
            psA = ctx.enter_context(tc.tile_pool(name="psA", bufs=2, space="PSUM"))
            psB = ctx.enter_context(tc.tile_pool(name="psB", bufs=4, space="PSUM"))
            psX = ctx.enter_context(tc.tile_pool(name="psX", bufs=2, space="PSUM"))

            def cload(name, ap, shape, dt=f32):
                t = const.tile(shape, dt, tag=name)
                nc.scalar.dma_start(t[:], ap)
                return t

            tz1w0 = cload("tz1w0", z1w0[:, :], [126, 32])
            tz1w1 = cload("tz1w1", z1w1[:, :], [90, 32])
            tz2w = [cload(f"tz2w{g}", z2w[g][:, :], [128, 64]) for g in range(4)]
            tz3w = [cload(f"tz3w{g}", z3w[g][:, :], [128, 96]) for g in range(4)]
            tqewT = cload("tqewT", qewT[:, :], [96, 128])
            twpT = cload("twpT", wpT[:, :], [128, 640])
            tc1b = cload("tc1b", c1b[:, :], [32, 1])
            tc2b = cload("tc2b", c2b[:, :], [64, 1])
            tc3b = cload("tc3b", c3b[:, :], [96, 1])
            tqeb = cload("tqeb", qeb[:, :], [128, 1])
            tident = cload("tident", ident[:, :], [128, 128])
            tkpad = cload("tkpad", kpad[:, :], [128, 2], u16)
            pooledacc = const.tile([96, BL], f32, tag="pooledacc")
            twpbr = cload("twpbr", wpbr[:, :], [128, 640]) if with_wpb else None

            for b in range(_DBG_BL):
                # ---- load x[b] as 14 shifted channel-major copies: Z1[9s+c, t] = x[b,0,t+s,c]
                with tc.high_priority(offset=300):
                    z1 = xp.tile([126, T], f32, tag="z1")
                    xt = xp.tile([128, 144], f32, tag="xt")
                    nc.gpsimd.dma_start(xt[:, :], xs[b, 0].rearrange("(p w) c -> p (w c)", p=128))
                    z1v = z1[0:9, :].rearrange("c (p tl) -> c tl p", tl=16)
                    for a in range(4):
                        pz = psX.tile([128, 512], f32, tag="psX")
                        for i in range(4):
                            tl = 4 * a + i
                            nc.tensor.transpose(pz[0:9, 128 * i:128 * i + 128],
                                            xt[:, 9 * tl:9 * tl + 9], tident[:, :])
                        nc.scalar.copy(z1v[:, 4 * a:4 * a + 4, :],
                                       pz[0:9, 0:512].rearrange("c (tl p) -> c tl p", p=128))
                    # shifted copies: rows 9s+c hold x[t+s, c]; s1-6 parallel, s7-13 block
                    for s in range(1, 7):
                        eng = nc.sync if s % 2 else nc.scalar
                        for c0 in (0, 1024):
                            c1 = min(c0 + 1024, T - s)
                            eng.dma_start(z1[9 * s:9 * s + 9, c0:c1], z1[0:9, c0 + s:c1 + s])
                    for c0 in (0, 680, 1360):
                        c1 = min(c0 + 680, T - 7)
                        nc.sync.dma_start(z1[63:126, c0:c1], z1[0:63, c0 + 7:c1 + 7])
                if _DBG_STAGES < 1:
                    continue
                # ---- conv1 -> relu -> Z2 rows 0:32 (cols 0:2025)
                z2 = wk.tile([128, T1], f32, tag="z2")
                for t0, n in _tiles(T1):
                    tc.cur_priority -= 150
                    ps = psA.tile([96, 512], f32, tag="psA")
                    nc.tensor.matmul(ps[0:32, 0:n], tz1w0[:, :], z1[0:126, t0:t0 + n],
                                     start=True, stop=False)
                    nc.tensor.matmul(ps[0:32, 0:n], tz1w1[:, :], z1[0:90, t0 + 14:t0 + 14 + n],
                                     start=False, stop=True)
                    nc.scalar.activation(z2[0:32, t0:t0 + n], ps[0:32, 0:n], RELU, bias=tc1b[:, :])
                    tc.cur_priority += 150
                if _DBG_STAGES < 2:
                    continue
                # shifted copies for conv2 stacking: Z2[32s+c, u] = relu1[c, u+s]
                for s in range(1, 4):
                    eng = nc.sync if s % 2 else nc.scalar
                    for c0 in (0, 680, 1360):
                        c1 = min(c0 + 680, T2 + 12)
                        eng.dma_start(z2[32 * s:32 * s + 32, c0:c1],
                                      z2[0:32, c0 + s:c1 + s])
                # ---- conv2 -> relu -> Z3 rows 0:64 (cols 0:2010)
                z3 = wk.tile([128, T2], f32, tag="z3")
                for t0, n in _tiles(T2):
                    tc.cur_priority -= 100
                    ps = psB.tile([128, 512], f32, tag="psB")
                    for g in range(4):
                        nc.tensor.matmul(ps[0:64, 0:n], tz2w[g][:, :],
                                         z2[:, t0 + 4 * g:t0 + 4 * g + n],
                                         start=(g == 0), stop=(g == 3))
                    nc.scalar.activation(z3[0:64, t0:t0 + n], ps[0:64, 0:n], RELU, bias=tc2b[:, :])
                    tc.cur_priority += 100
                for c0 in (0, 640, 1280, 1920):
                    c1 = min(c0 + 640, T3 + 6)
                    nc.sync.dma_start(z3[64:128, c0:c1], z3[0:64, c0 + 1:c1 + 1])
                if _DBG_STAGES < 3:
                    continue
                # ---- conv3 -> relu -> h3 [96, 2003]
                h3 = wk.tile([96, TPAD], f32, tag="h3")
                for t0, n in _tiles(T3):
                    tc.cur_priority -= 60
                    ps = psA.tile([96, 512], f32, tag="psA")
                    for g in range(4):
                        nc.tensor.matmul(ps[0:96, 0:n], tz3w[g][:, :],
                                         z3[:, t0 + 2 * g:t0 + 2 * g + n],
                                         start=(g == 0), stop=(g == 3))
                    nc.scalar.activation(h3[0:96, t0:t0 + n], ps[0:96, 0:n], RELU, bias=tc3b[:, :])
                    tc.cur_priority += 60
                if _DBG_STAGES < 4:
                    continue
                # ---- qe -> relu -> q [128, 2048] (tail zeroed)
                q = wk.tile([128, TPAD], f32, tag="q")
                nc.gpsimd.memset(q[:, T3:TPAD], 0.0)
                for t0, n in _tiles(T3):
                    ps = psB.tile([128, 512], f32, tag="psB")
                    nc.tensor.matmul(ps[:, 0:n], tqewT[:, :], h3[0:96, t0:t0 + n],
                                     start=True, stop=True)
                    nc.scalar.activation(q[:, t0:t0 + n], ps[:, 0:n], RELU, bias=tqeb[:, :])
                if _DBG_STAGES < 5:
                    continue
                # ---- wp logits (t-major) + exact argmax per group
                kst = wk.tile([128, 2 * NT], u16, tag="kst")
                nc.vector.tensor_copy(kst[64:128, 2 * (NT - 1):2 * NT], tkpad[64:128, :])
                for j in range(NT):
                    t0 = 128 * j
                    nv = 128 if j < NT - 1 else LASTN
                    psl = psB.tile([128, 512], f32, tag="psB")
                    psh = psB.tile([128, 512], f32, tag="psB")
                    nc.tensor.matmul(psl[:, 0:512], q[:, t0:t0 + 128], twpT[:, 0:512],
                                     start=True, stop=True)
                    nc.tensor.matmul(psh[:, 0:128], q[:, t0:t0 + 128], twpT[:, 512:640],
                                     start=True, stop=True)
                    ls = lsp.tile([128, 640], f32, tag="ls")
                    nc.scalar.copy(ls[:, 0:512], psl[:, 0:512])
                    nc.scalar.copy(ls[:, 512:640], psh[:, 0:128])
                    if with_wpb:
                        nc.vector.tensor_add(ls[:, :], ls[:, :], twpbr[:, :])
                    mx = sm.tile([128, 16], f32, tag="mx")
                    nc.vector.max(mx[:, 0:8], ls[:, 0:V])
                    nc.vector.max(mx[:, 8:16], ls[:, V:2 * V])
                    ki = sm.tile([128, 16], u16, tag="ki")
                    nc.vector.max_index(ki[:, 0:8], mx[:, 0:8], ls[:, 0:V])
                    nc.vector.max_index(ki[:, 8:16], mx[:, 8:16], ls[:, V:2 * V])
                    kiv = ki[0:nv, :].rearrange("p (a b) -> p a b", b=8)[:, :, 0]
                    kdst = kst[:].rearrange("p (j g) -> p j g", g=2)[0:nv, j, :]
                    nc.vector.tensor_copy(kdst, kiv)
                # ---- targets (int32) out
                kt32 = lsp.tile([128, 2 * NT], i32, tag="kt32")
                nc.vector.tensor_copy(kt32[:, :], kst[:, :])
                tv = kt32[:].rearrange("p (j g) -> p j g", g=2)
                nc.gpsimd.dma_start(
                    targets[b, 0:1920, :].rearrange("(j p) g -> p j g", p=128),
                    tv[:, 0:15, :])
                nc.gpsimd.dma_start(targets[b, 1920:T3, :], tv[0:LASTN, 15, :])
                if _DBG_STAGES < 6:
                    continue
                # ---- k -> DRAM -> wrapped/replicated gather indices
                nc.sync.dma_start(
                    kdram[b].rearrange("(j p) g -> p j g", p=128),
                    kst[:].rearrange("p (j g) -> p j g", g=2))
                if _DBG_SUB < 1:
                    continue
                idx0 = wk.tile([32, TPAD // 16], i16, tag="idx0")
                idx1 = wk.tile([32, TPAD // 16], i16, tag="idx1")
                ksrc = kdram[b].bitcast(i16).rearrange("(s q) g -> q s g", q=16)
                nc.sync.dma_start(idx0[0:16, :], ksrc[:, :, 0])
                nc.sync.dma_start(idx1[0:16, :], ksrc[:, :, 1])
                nc.sync.dma_start(idx0[16:32, :], idx0[0:16, :])
                nc.sync.dma_start(idx1[16:32, :], idx1[0:16, :])
                # ---- gathers: quant rows + de product-table rows
                if _DBG_SUB < 2:
                    continue
                qg0 = wk.tile([128, NT, VD], f32, tag="qg0")
                qg1 = wk.tile([128, NT, VD], f32, tag="qg1")
                dg0 = dgp.tile([128, NT, 128], f32, tag="dg0")
                dg1 = dgp.tile([128, NT, 128], f32, tag="dg1")
                nc.gpsimd.dma_gather(qg0[:], cbq[:, :], idx0[:, :], num_idxs=TPAD,
                                     num_idxs_reg=TPAD, elem_size=VD, single_packet=False)
                if _DBG_SUB >= 3:
                    nc.gpsimd.dma_gather(qg1[:], cbq[V:641, :], idx1[:, :], num_idxs=TPAD,
                                         num_idxs_reg=TPAD, elem_size=VD, single_packet=False)
                else:
                    nc.gpsimd.dma_gather(qg1[:], cbq[0:321, :], idx1[:, :], num_idxs=TPAD,
                                         num_idxs_reg=TPAD, elem_size=VD, single_packet=False)
                if _DBG_SUB >= 4:
                    nc.gpsimd.dma_gather(dg0[:], p0t[:, :], idx0[:, :], num_idxs=TPAD,
                                         num_idxs_reg=TPAD, elem_size=128, single_packet=False)
                    nc.gpsimd.dma_gather(dg1[:], p1t[:, :], idx1[:, :], num_idxs=TPAD,
                                         num_idxs_reg=TPAD, elem_size=128, single_packet=False)
                else:
                    nc.gpsimd.memset(dg0[:], 0.0)
                    nc.gpsimd.memset(dg1[:], 0.0)
                if _DBG_SUB < 5:
                    continue
                # ---- quant out (row t lives at partition t%128, slot t//128)
                nc.gpsimd.dma_start(
                    quant[b, 0:1920, 0:64].rearrange("(s p) c -> p s c", p=128),
                    qg0[:, 0:15, :])
                nc.gpsimd.dma_start(
                    quant[b, 0:1920, 64:128].rearrange("(s p) c -> p s c", p=128),
                    qg1[:, 0:15, :])
                nc.gpsimd.dma_start(quant[b, 1920:T3, 0:64], qg0[0:LASTN, 15, :])
                nc.gpsimd.dma_start(quant[b, 1920:T3, 64:128], qg1[0:LASTN, 15, :])
                # ---- de (gathered product rows) -> pooled
                with tc.high_priority(offset=-520):
                    nc.vector.tensor_add(dg0[:], dg0[:], dg1[:])
                    dmax = dpool.tile([128, 128], f32, tag="dmax")
                    nc.vector.reduce_max(dmax[:, :], dg0[:].rearrange("p s c -> p c s"), axis=X)
                    dmax_tiles.append(dmax)
            for b, dmax in enumerate(dmax_tiles):
                with tc.high_priority(offset=max(0, (5 - b)) * 290):
                    pst = psX.tile([128, 512], f32, tag="psX")
                    nc.tensor.transpose(pst[:, 0:128], dmax[:, :], tident[:, :])
                    poo = sm.tile([128, 1], f32, tag="poo")
                    nc.vector.reduce_max(poo[:, :], pst[:, 0:128], axis=X)
                    nc.vector.tensor_scalar_max(pooledacc[0:96, b:b + 1], poo[0:96, :], 0.0)
            nc.scalar.dma_start(pooled[:, :].rearrange("b c -> c b"), pooledacc[:, :])

    nc.compile()
    return nc


def _prep_consts(c1_w, c1_b, c2_w, c2_b, c3_w, c3_b, qe_w, qe_b, wp_w, wp_b,
                 codebook, de_w, de_b):
    f = np.float32
    c1t = np.ascontiguousarray(c1_w.transpose(2, 1, 0))      # [24, 9, 32]
    c2t = np.ascontiguousarray(c2_w.transpose(2, 1, 0))      # [16, 32, 64]
    c3t = np.ascontiguousarray(c3_w.transpose(2, 1, 0))      # [8, 64, 96]
    cb = np.ascontiguousarray(codebook[0]).astype(f)         # [640, 64]
    consts = {
        "ident": np.eye(128, dtype=f),
        "z1w0": np.ascontiguousarray(c1t[0:14].reshape(126, 32)).astype(f),
        "z1w1": np.ascontiguousarray(c1t[14:24].reshape(90, 32)).astype(f),
        "qewT": np.ascontiguousarray(qe_w.T).astype(f),
        "wpT": np.ascontiguousarray(wp_w.T).astype(f),
        "c1b": c1_b.reshape(32, 1).astype(f),
        "c2b": c2_b.reshape(64, 1).astype(f),
        "c3b": c3_b.reshape(96, 1).astype(f),
        "qeb": qe_b.reshape(128, 1).astype(f),
        "kpad": np.full((128, 2), V, dtype=np.uint16),
    }
    for g in range(4):
        consts[f"z2w{g}"] = np.ascontiguousarray(c2t[4 * g:4 * g + 4].reshape(128, 64)).astype(f)
        consts[f"z3w{g}"] = np.ascontiguousarray(c3t[2 * g:2 * g + 2].reshape(128, 96)).astype(f)
    consts["cbq"] = np.concatenate([cb, np.zeros((1, VD), f)], axis=0)
    p0 = np.zeros((321, 128), f)
    p0[0:320, 0:96] = cb[0:320] @ de_w[:, 0:64].T + de_b[None, :]
    p0[320, :] = -1e30
    p1 = np.zeros((321, 128), f)
    p1[0:320, 0:96] = cb[320:640] @ de_w[:, 64:128].T
    p1[320, :] = -1e30
    consts["p0t"] = p0
    consts["p1t"] = p1
    return consts


def kernel(x, c1_w, c1_b, c2_w, c2_b, c3_w, c3_b, qe_w, qe_b, wp_w, wp_b,
           codebook, de_w, de_b):
    from concourse import bass_utils

    args = [np.asarray(a) for a in (x, c1_w, c1_b, c2_w, c2_b, c3_w, c3_b,
                                    qe_w, qe_b, wp_w, wp_b, codebook, de_w, de_b)]
    (x, c1_w, c1_b, c2_w, c2_b, c3_w, c3_b, qe_w, qe_b, wp_w, wp_b,
     codebook, de_w, de_b) = args
    with_wpb = bool(np.any(wp_b != 0))
    if with_wpb not in _built:
        _built[with_wpb] = _build(with_wpb)
    nc = _built[with_wpb]

    consts = _prep_consts(c1_w, c1_b, c2_w, c2_b, c3_w, c3_b, qe_w, qe_b,
                          wp_w, wp_b, codebook, de_w, de_b)
    if with_wpb:
        consts["wpbr"] = np.broadcast_to(wp_b.astype(np.float32), (128, 640)).copy()
    in_maps = []
    for i in range(NCORES):
        m = dict(consts)
        m["xs"] = np.ascontiguousarray(x[i * BL:(i + 1) * BL]).astype(np.float32)
        in_maps.append(m)

    res = bass_utils.run_bass_kernel_spmd(nc, in_maps, core_ids=list(range(NCORES)))

    pooled = np.concatenate([res.results[i]["pooled"] for i in range(NCORES)], axis=0)
    quant = np.concatenate([res.results[i]["quant"] for i in range(NCORES)], axis=0)
    targets = np.concatenate([res.results[i]["targets"] for i in range(NCORES)], axis=0)
    return pooled, quant, targets.astype(np.int32)
